# revision 12
# baseline (speedup 1.0000x reference)
"""EuclideanCodebook (VQ) kernel for 8x TRN2 NeuronCores.

Data-parallel over tokens; embed replicated; per-chunk ReduceScatter of the
segment sums (embed_sum cols 0..D-1, bins col D) before the EMA update.

Self-contained: hardcodes problem geometry from the spec.
"""

import sys

sys.path.insert(0, "/opt/trn_rl_repo")

import numpy as np
from contextlib import ExitStack

import concourse.bass as bass
import concourse.mybir as mybir
import concourse.tile as tile
from concourse import bacc
from concourse import bass_utils
from concourse import bass_isa
from concourse.masks import make_identity

F32 = mybir.dt.float32
F32R = mybir.dt.float32r
BF16 = mybir.dt.bfloat16
I32 = mybir.dt.int32
U32 = mybir.dt.uint32
U8 = mybir.dt.uint8
P = 128

DECAY = 0.8
EPS = 1e-5


class Cfg:
    def __init__(self, cores=8, ntok=4096, C=8192, D=512):
        self.CORES = cores
        self.NTOK = ntok            # tokens per core
        self.C = C                  # codebook size
        self.D = D                  # embedding dim
        self.TILES = ntok // P      # token tiles per core
        self.DB = D // P            # d sub-blocks (4)
        self.NSEG = 4               # phase-A codebook segments resident in SBUF
        self.CSEG = C // self.NSEG
        self.CBA = min(512, self.CSEG)       # phase-A c-block (psum N)
        self.NCBA = self.CSEG // self.CBA    # c-blocks per segment
        self.NCHUNK = 8             # ReduceScatter chunks
        self.CHUNK = C // self.NCHUNK
        self.ROWS = self.CHUNK // cores      # c rows per core per chunk
        self.CBB = min(512, self.CHUNK)      # phase-B c-block (psum N)
        self.NCBB = self.CHUNK // self.CBB   # phase-B c-blocks per chunk
        self.NTOT = ntok * cores    # global token count
        assert self.ROWS <= 128 and self.CSEG % self.CBA == 0
        assert self.CHUNK % self.CBB == 0 and ntok % P == 0
        assert self.C % (self.NSEG * P) == 0


def build_nc(cfg: Cfg, mm_dt=F32R, debug=False, with_collectives=True):
    """Build the SPMD program (identical on every core)."""
    c = cfg
    SD = c.D + 1  # staged row: D embed_sum cols + 1 bins col

    nc = bacc.Bacc(
        "TRN2",
        target_bir_lowering=False,
        debug=debug,
        num_devices=c.CORES,
    )

    # ---- kernel I/O (per core) ----
    x_d = nc.dram_tensor("x_sh", [c.NTOK, c.D], F32, kind="ExternalInput")
    e_d = nc.dram_tensor("embed", [c.C, c.D], F32, kind="ExternalInput")
    csf_d = nc.dram_tensor("cs_full", [c.C // P, P], F32, kind="ExternalInput")
    cso_d = nc.dram_tensor("cs_own", [c.NCHUNK * c.ROWS, 1], F32, kind="ExternalInput")
    eao_d = nc.dram_tensor("ea_own", [c.NCHUNK * c.ROWS, c.D], F32, kind="ExternalInput")

    dist_d = nc.dram_tensor("dist_sh", [c.NTOK, c.C], F32, kind="ExternalOutput")
    ei_d = nc.dram_tensor("embed_ind_sh", [c.NTOK], I32, kind="ExternalOutput")
    q_d = nc.dram_tensor("quantize_sh", [c.NTOK, c.D], F32, kind="ExternalOutput")
    en_d = nc.dram_tensor("embed_new_sh", [c.NCHUNK * c.ROWS, c.D], F32, kind="ExternalOutput")
    csn_d = nc.dram_tensor("cs_new_sh", [c.NCHUNK * c.ROWS, 1], F32, kind="ExternalOutput")
    ean_d = nc.dram_tensor("ea_new_sh", [c.NCHUNK * c.ROWS, c.D], F32, kind="ExternalOutput")

    # ---- internal DRAM for collectives ----
    rs_in = [
        nc.dram_tensor(f"rs_in{m}", [c.CHUNK, SD], F32, kind="Internal")
        for m in range(c.NCHUNK)
    ]
    rs_out = [
        nc.dram_tensor(f"rs_out{m}", [c.ROWS, SD], F32, kind="Internal")
        for m in range(c.NCHUNK)
    ]

    groups = [list(range(c.CORES))]

    def r32(ap):
        return ap.bitcast(mm_dt)

    with ExitStack() as ctx:
        tc = ctx.enter_context(tile.TileContext(nc))

        cpool = ctx.enter_context(tc.tile_pool(name="cpool", bufs=1))
        epool = ctx.enter_context(tc.tile_pool(name="epool", bufs=3))
        etpool = ctx.enter_context(tc.tile_pool(name="etpool", bufs=1))
        xpool = ctx.enter_context(tc.tile_pool(name="xpool", bufs=2))
        wpool = ctx.enter_context(tc.tile_pool(name="wpool", bufs=2))
        dpool = ctx.enter_context(tc.tile_pool(name="dpool", bufs=2))
        mpool = ctx.enter_context(tc.tile_pool(name="mpool", bufs=1))
        bpool = ctx.enter_context(tc.tile_pool(name="bpool", bufs=3))
        sbacc = ctx.enter_context(tc.tile_pool(name="sbacc", bufs=1))
        stpool = ctx.enter_context(tc.tile_pool(name="stpool", bufs=2))
        empool = ctx.enter_context(tc.tile_pool(name="empool", bufs=2))

        spsum = ctx.enter_context(tc.tile_pool(name="spsum", bufs=2, space="PSUM"))
        tpsum = ctx.enter_context(tc.tile_pool(name="tpsum", bufs=1, space="PSUM"))
        apsum = ctx.enter_context(tc.tile_pool(name="apsum", bufs=1, space="PSUM"))

        # ================= constants =================
        ident = cpool.tile([P, P], F32)
        make_identity(nc, ident[:])

        iota_i = cpool.tile([P, c.CBB], I32)
        nc.gpsimd.iota(iota_i[:], pattern=[[1, c.CBB]], base=0, channel_multiplier=0)
        iotaf = cpool.tile([P, c.CBB], F32)
        nc.vector.tensor_copy(iotaf[:], iota_i[:])

        onn32 = cpool.tile([1, P], F32)
        nc.vector.memset(onn32[:], -0.5)
        oneneg = cpool.tile([1, P], F32R)   # lhsT for the -0.5*y2 bias matmul
        nc.scalar.copy(out=oneneg[:], in_=onn32[:])
        onecol = cpool.tile([P, 1], BF16)  # lhsT for the bins matmul
        nc.vector.memset(onecol[:], 1.0)
        junk = cpool.tile([P, c.D], F32)   # write-only sink for Square outputs

        # ================= alpha (laplace smoothing scalars) =================
        csm = cpool.tile([c.C // P, P], F32)
        nc.sync.dma_start(out=csm[:], in_=csf_d[:, :])
        csr = cpool.tile([c.C // P, 1], F32)
        nc.vector.reduce_sum(csr[:], csm[:], axis=mybir.AxisListType.X)
        csra = cpool.tile([c.C // P, 1], F32)
        nc.gpsimd.partition_all_reduce(
            csra[:], csr[:], channels=c.C // P, reduce_op=bass_isa.ReduceOp.add
        )
        total = cpool.tile([1, 1], F32)
        # total = DECAY * sum(cs) + (1-DECAY) * NTOT   (sum of bins == NTOT)
        nc.vector.tensor_scalar(
            out=total[:], in0=csra[0:1, :], scalar1=DECAY,
            scalar2=(1.0 - DECAY) * float(c.NTOT),
            op0=mybir.AluOpType.mult, op1=mybir.AluOpType.add,
        )
        denom = cpool.tile([1, 1], F32)
        nc.vector.tensor_scalar_add(denom[:], total[:], float(c.C) * EPS)
        dinv = cpool.tile([1, 1], F32)
        nc.vector.reciprocal(dinv[:], denom[:])
        alpha = cpool.tile([1, 1], F32)
        nc.vector.tensor_tensor(alpha[:], total[:], dinv[:], op=mybir.AluOpType.mult)
        acol = cpool.tile([P, 1], F32)
        nc.gpsimd.partition_broadcast(acol[:], alpha[:])
        aepscol = cpool.tile([P, 1], F32)
        nc.vector.tensor_scalar_mul(aepscol[:], acol[:], EPS)

        # x in bf16, resident for phase B
        xbf = cpool.tile([P, c.TILES * c.D], BF16)

        # ================= phase A =================
        # per-segment top-2 (value + index as f32) per token, interleaved per tile
        mq = [mpool.tile([P, 2 * c.TILES], F32, name=f"mq{s}") for s in range(c.NSEG)]
        iq = [mpool.tile([P, 2 * c.TILES], F32, name=f"iq{s}") for s in range(c.NSEG)]

        for q in range(c.NSEG):
            c0 = q * c.CSEG
            # ---- build embedT (d-major) + y2 for this segment ----
            emt = [
                etpool.tile([P, c.CSEG], F32R, name=f"emt{k}", tag=f"emt{k}")
                for k in range(c.DB)
            ]
            y2col = etpool.tile([P, c.CSEG // P], F32, tag="y2col")
            for ct in range(c.CSEG // P):
                e_t = epool.tile([P, c.D], F32, tag="e_t")
                nc.sync.dma_start(out=e_t[:], in_=e_d[c0 + ct * P:c0 + (ct + 1) * P, :])
                nc.scalar.activation(
                    junk[:], e_t[:], mybir.ActivationFunctionType.Square,
                    accum_out=y2col[:, ct:ct + 1],
                )
                for k in range(c.DB):
                    tp = tpsum.tile([P, P], F32, tag="tp")
                    nc.tensor.transpose(tp[:], e_t[:, k * P:(k + 1) * P], ident[:])
                    nc.scalar.copy(out=emt[k][:, ct * P:(ct + 1) * P], in_=tp[:])
            # y2 row layout: (128, CSEG//128) -> transpose -> (CSEG//128, 128) -> row
            y2row = etpool.tile([1, c.CSEG], F32R, tag="y2row")
            ytp = tpsum.tile([P, P], F32, tag="tp")
            nc.tensor.transpose(ytp[:c.CSEG // P, :], y2col[:], ident[:])
            y2tr = etpool.tile([c.CSEG // P, P], F32R, tag="y2tr")
            nc.scalar.copy(out=y2tr[:], in_=ytp[:c.CSEG // P, :])
            nc.sync.dma_start(
                out=y2row[0:1, :].rearrange("one (a b) -> one a b", b=P),
                in_=y2tr[:],
            )

            # ---- token tiles ----
            for t in range(c.TILES):
                x_t = xpool.tile([P, c.D], F32, tag="x_t")
                nc.sync.dma_start(out=x_t[:], in_=x_d[t * P:(t + 1) * P, :])
                x2c = xpool.tile([P, 1], F32, tag="x2c")
                nc.scalar.activation(
                    junk[:], x_t[:], mybir.ActivationFunctionType.Square,
                    accum_out=x2c[:],
                )
                if q == 0:
                    nc.vector.tensor_copy(xbf[:, t * c.D:(t + 1) * c.D], x_t[:])
                xt = xpool.tile([P, c.D], F32R, tag="xt")  # x^T: [d-part, n] blocks
                for k in range(c.DB):
                    tp = tpsum.tile([P, P], F32, tag="tp")
                    nc.tensor.transpose(tp[:], x_t[:, k * P:(k + 1) * P], ident[:])
                    nc.scalar.copy(out=xt[:, k * P:(k + 1) * P], in_=tp[:])

                dseg = dpool.tile([P, c.CSEG], F32, tag="dseg")
                for cb in range(c.NCBA):
                    s_ps = spsum.tile([P, c.CBA], F32, tag="s")
                    for k in range(c.DB):
                        nc.tensor.matmul(
                            s_ps[:],
                            lhsT=xt[:, k * P:(k + 1) * P],
                            rhs=emt[k][:, cb * c.CBA:(cb + 1) * c.CBA],
                            start=(k == 0), stop=False,
                        )
                    nc.tensor.matmul(
                        s_ps[:],
                        lhsT=oneneg[:],
                        rhs=y2row[0:1, cb * c.CBA:(cb + 1) * c.CBA],
                        start=False, stop=True,
                    )
                    # r = relu(-2*s + x2) = ||x-e||^2 ; sq = sqrt(r); dist = -sq
                    r_t = wpool.tile([P, c.CBA], F32, tag="r_t")
                    nc.scalar.activation(
                        r_t[:], s_ps[:], mybir.ActivationFunctionType.Relu,
                        bias=x2c[:], scale=-2.0,
                    )
                    sq_t = wpool.tile([P, c.CBA], F32, tag="sq_t")
                    nc.scalar.activation(
                        sq_t[:], r_t[:], mybir.ActivationFunctionType.Sqrt,
                    )
                    nc.gpsimd.tensor_scalar_mul(
                        dseg[:, cb * c.CBA:(cb + 1) * c.CBA], sq_t[:], -1.0
                    )
                nc.sync.dma_start(
                    out=dist_d[t * P:(t + 1) * P, c0:c0 + c.CSEG], in_=dseg[:]
                )
                m8 = wpool.tile([P, 8], F32, tag="m8")
                nc.vector.max(m8[:], dseg[:])
                i8 = wpool.tile([P, 8], U32, tag="i8")
                nc.vector.max_index(i8[:], m8[:], dseg[:])
                nc.vector.tensor_copy(mq[q][:, 2 * t:2 * t + 2], m8[:, 0:2])
                nc.vector.tensor_copy(iq[q][:, 2 * t:2 * t + 2], i8[:, 0:2])

        # ---- merge segments via exact top-2 rescore ----
        # Candidates: per-segment top-2 under f32r scores. Pick global top-2,
        # recompute their exact fp32 ||x-e||^2 and choose (ties -> lower idx).
        iqg = [mpool.tile([P, 2 * c.TILES], F32, name=f"iqg{s}") for s in range(c.NSEG)]
        for s in range(c.NSEG):
            nc.vector.tensor_scalar_add(iqg[s][:], iq[s][:], float(s * c.CSEG))
        idxf = mpool.tile([P, c.TILES], F32)
        rpool = ctx.enter_context(tc.tile_pool(name="rpool", bufs=2))
        NCAND = 2 * c.NSEG
        for t in range(c.TILES):
            cm = rpool.tile([P, NCAND], F32, tag="cm")
            ci = rpool.tile([P, NCAND], F32, tag="ci")
            for s in range(c.NSEG):
                nc.vector.tensor_copy(cm[:, 2 * s:2 * s + 2], mq[s][:, 2 * t:2 * t + 2])
                nc.vector.tensor_copy(ci[:, 2 * s:2 * s + 2], iqg[s][:, 2 * t:2 * t + 2])
            s8 = rpool.tile([P, 8], F32, tag="s8")
            nc.vector.max(s8[:], cm[:])
            p8 = rpool.tile([P, 8], U32, tag="p8")
            nc.vector.max_index(p8[:], s8[:], cm[:])
            p8f = rpool.tile([P, 8], F32, tag="p8f")
            nc.vector.tensor_copy(p8f[:], p8[:])
            xr = rpool.tile([P, c.D], F32, tag="xr")
            nc.sync.dma_start(out=xr[:], in_=x_d[t * P:(t + 1) * P, :])
            iand = []
            d2 = []
            for j in range(2):
                sel = rpool.tile([P, NCAND], F32, tag=f"sel{j}")
                nc.vector.scalar_tensor_tensor(
                    out=sel[:], in0=iotaf[:, :NCAND], scalar=p8f[:, j:j + 1],
                    in1=ci[:], op0=mybir.AluOpType.is_equal,
                    op1=mybir.AluOpType.mult,
                )
                icf = rpool.tile([P, 1], F32, tag=f"icf{j}")
                nc.vector.reduce_sum(icf[:], sel[:], axis=mybir.AxisListType.X)
                ici = rpool.tile([P, 1], I32, tag=f"ici{j}")
                nc.vector.tensor_copy(ici[:], icf[:])
                iand.append((icf, ici))
                g_j = rpool.tile([P, c.D], F32, tag=f"g{j}")
                nc.gpsimd.indirect_dma_start(
                    out=g_j[:], out_offset=None, in_=e_d[:],
                    in_offset=bass.IndirectOffsetOnAxis(ap=ici[:, 0:1], axis=0),
                )
                df = rpool.tile([P, c.D], F32, tag=f"df{j}")
                nc.vector.tensor_tensor(df[:], xr[:], g_j[:],
                                        op=mybir.AluOpType.subtract)
                nc.vector.tensor_tensor(df[:], df[:], df[:],
                                        op=mybir.AluOpType.mult)
                rr = rpool.tile([P, c.DB], F32, tag=f"rr{j}")
                nc.vector.reduce_sum(
                    rr[:], df[:].rearrange("p (a b) -> p a b", b=P),
                    axis=mybir.AxisListType.X,
                )
                d2j = rpool.tile([P, 1], F32, tag=f"d2{j}")
                nc.vector.reduce_sum(d2j[:], rr[:], axis=mybir.AxisListType.X)
                d2.append(d2j)
            lt = rpool.tile([P, 1], U8, tag="lt")
            nc.vector.tensor_tensor(lt[:], d2[0][:], d2[1][:],
                                    op=mybir.AluOpType.is_lt)
            eqm = rpool.tile([P, 1], U8, tag="eqm")
            nc.vector.tensor_tensor(eqm[:], d2[0][:], d2[1][:],
                                    op=mybir.AluOpType.is_equal)
            imin = rpool.tile([P, 1], F32, tag="imin")
            nc.vector.tensor_tensor(imin[:], iand[0][0][:], iand[1][0][:],
                                    op=mybir.AluOpType.min)
            tmpi = rpool.tile([P, 1], F32, tag="tmpi")
            nc.vector.select(tmpi[:], mask=lt[:], on_true=iand[0][0][:],
                             on_false=iand[1][0][:])
            nc.vector.select(idxf[:, t:t + 1], mask=eqm[:], on_true=imin[:],
                             on_false=tmpi[:])
        idxi = mpool.tile([P, c.TILES], I32)
        nc.vector.tensor_copy(idxi[:], idxf[:])

        # ---- embed_ind output ----
        eit = tpsum.tile([P, P], F32, tag="tp")
        nc.tensor.transpose(eit[:c.TILES, :], idxf[:], ident[:])
        eisb = mpool.tile([c.TILES, P], I32)
        nc.vector.tensor_copy(eisb[:], eit[:c.TILES, :])
        nc.sync.dma_start(
            out=ei_d[:].rearrange("(t p) -> t p", p=P), in_=eisb[:]
        )

        # ---- quantize: gather embed rows ----
        for t in range(c.TILES):
            q_t = bpool.tile([P, c.D], F32, tag="q_t")
            nc.gpsimd.indirect_dma_start(
                out=q_t[:],
                out_offset=None,
                in_=e_d[:],
                in_offset=bass.IndirectOffsetOnAxis(ap=idxi[:, t:t + 1], axis=0),
            )
            nc.sync.dma_start(out=q_d[t * P:(t + 1) * P, :], in_=q_t[:])

        # ================= phase B: segment sums + ReduceScatter =================
        for m in range(c.NCHUNK):
            for cbl in range(c.NCBB):
                cb0 = m * c.CHUNK + cbl * c.CBB  # global c offset of this block
                idsh = bpool.tile([P, c.TILES], F32, tag="idsh")
                nc.vector.tensor_scalar_add(idsh[:], idxf[:], -float(cb0))
                acc = [
                    apsum.tile([P, c.CBB], F32, name=f"acc{k}", tag=f"acc{k}")
                    for k in range(c.DB)
                ]
                accb = apsum.tile([1, c.CBB], F32, tag="accb")
                for t in range(c.TILES):
                    oh = bpool.tile([P, c.CBB], BF16, tag="oh")
                    nc.vector.tensor_scalar(
                        out=oh[:], in0=iotaf[:, :c.CBB], scalar1=idsh[:, t:t + 1],
                        scalar2=None, op0=mybir.AluOpType.is_equal,
                    )
                    for k in range(c.DB):
                        nc.tensor.matmul(
                            acc[k][:],
                            lhsT=xbf[:, t * c.D + k * P:t * c.D + (k + 1) * P],
                            rhs=oh[:],
                            start=(t == 0), stop=(t == c.TILES - 1),
                        )
                    nc.tensor.matmul(
                        accb[:],
                        lhsT=onecol[:],
                        rhs=oh[:],
                        start=(t == 0), stop=(t == c.TILES - 1),
                    )
                # copy psum accumulators to sbuf
                asb = [
                    sbacc.tile([P, c.CBB], F32, name=f"asb{k}", tag=f"asb{k}")
                    for k in range(c.DB)
                ]
                absb = sbacc.tile([1, c.CBB], F32, tag="absb")
                for k in range(c.DB):
                    nc.scalar.copy(out=asb[k][:], in_=acc[k][:])
                nc.vector.tensor_copy(absb[:], accb[:])
                # transpose to c-major and stage
                for j in range(c.CBB // P):
                    stg = stpool.tile([P, SD], F32, tag="stg")
                    for k in range(c.DB):
                        tp = tpsum.tile([P, P], F32, tag="tp")
                        nc.tensor.transpose(tp[:], asb[k][:, j * P:(j + 1) * P], ident[:])
                        nc.scalar.copy(out=stg[:, k * P:(k + 1) * P], in_=tp[:])
                    tpb = tpsum.tile([P, P], F32, tag="tp")
                    nc.tensor.transpose(
                        tpb[:, 0:1], absb[0:1, j * P:(j + 1) * P], ident[0:1, 0:1]
                    )
                    nc.scalar.copy(out=stg[:, c.D:SD], in_=tpb[:, 0:1])
                    nc.sync.dma_start(
                        out=rs_in[m][cbl * c.CBB + j * P:cbl * c.CBB + (j + 1) * P, :],
                        in_=stg[:],
                    )
            if with_collectives:
                nc.gpsimd.collective_compute(
                    "ReduceScatter",
                    mybir.AluOpType.add,
                    replica_groups=groups,
                    ins=[rs_in[m].ap().opt()],
                    outs=[rs_out[m].ap().opt()],
                )
            else:
                # timing-model stub: keeps the dataflow shape without comms
                nc.sync.dma_start(out=rs_out[m][:, :], in_=rs_in[m][0:c.ROWS, :])

        # ================= EMA update on owned c rows =================
        for m in range(c.NCHUNK):
            es = empool.tile([c.ROWS, SD], F32, tag="es")
            nc.sync.dma_start(out=es[:], in_=rs_out[m][:, :])
            ea = empool.tile([c.ROWS, c.D], F32, tag="ea")
            nc.sync.dma_start(out=ea[:], in_=eao_d[m * c.ROWS:(m + 1) * c.ROWS, :])
            csc = empool.tile([c.ROWS, 1], F32, tag="csc")
            nc.sync.dma_start(out=csc[:], in_=cso_d[m * c.ROWS:(m + 1) * c.ROWS, :])

            cs08 = empool.tile([c.ROWS, 1], F32, tag="cs08")
            nc.vector.tensor_scalar_mul(cs08[:], csc[:], DECAY)
            csn = empool.tile([c.ROWS, 1], F32, tag="csn")
            nc.vector.scalar_tensor_tensor(
                out=csn[:], in0=es[:, c.D:SD], scalar=1.0 - DECAY, in1=cs08[:],
                op0=mybir.AluOpType.mult, op1=mybir.AluOpType.add,
            )
            ea08 = empool.tile([c.ROWS, c.D], F32, tag="ea08")
            nc.vector.tensor_scalar_mul(ea08[:], ea[:], DECAY)
            ean = empool.tile([c.ROWS, c.D], F32, tag="ean")
            nc.vector.scalar_tensor_tensor(
                out=ean[:], in0=es[:, :c.D], scalar=1.0 - DECAY, in1=ea08[:],
                op0=mybir.AluOpType.mult, op1=mybir.AluOpType.add,
            )
            smo = empool.tile([c.ROWS, 1], F32, tag="smo")
            nc.vector.scalar_tensor_tensor(
                out=smo[:], in0=csn[:], scalar=acol[:c.ROWS, :], in1=aepscol[:c.ROWS, :],
                op0=mybir.AluOpType.mult, op1=mybir.AluOpType.add,
            )
            sinv = empool.tile([c.ROWS, 1], F32, tag="sinv")
            nc.vector.reciprocal(sinv[:], smo[:])
            en = empool.tile([c.ROWS, c.D], F32, tag="en")
            nc.vector.tensor_scalar_mul(en[:], ean[:], sinv[:])

            nc.sync.dma_start(out=en_d[m * c.ROWS:(m + 1) * c.ROWS, :], in_=en[:])
            nc.sync.dma_start(out=csn_d[m * c.ROWS:(m + 1) * c.ROWS, :], in_=csn[:])
            nc.sync.dma_start(out=ean_d[m * c.ROWS:(m + 1) * c.ROWS, :], in_=ean[:])

    nc.compile()
    return nc


# ======================= host-side glue =======================

def owned_rows(cfg: Cfg, r: int) -> np.ndarray:
    """Global c indices owned by core r (concatenated per RS chunk)."""
    return np.concatenate([
        np.arange(m * cfg.CHUNK + r * cfg.ROWS, m * cfg.CHUNK + (r + 1) * cfg.ROWS)
        for m in range(cfg.NCHUNK)
    ])


def make_in_maps(cfg: Cfg, x, embed, cluster_size, embed_avg):
    """x: (n, d) full; embed: (C, D); cluster_size: (C,); embed_avg: (C, D)."""
    in_maps = []
    for r in range(cfg.CORES):
        rows = owned_rows(cfg, r)
        in_maps.append({
            "x_sh": np.ascontiguousarray(x[r * cfg.NTOK:(r + 1) * cfg.NTOK]),
            "embed": np.ascontiguousarray(embed),
            "cs_full": np.ascontiguousarray(cluster_size.reshape(cfg.C // P, P)),
            "cs_own": np.ascontiguousarray(cluster_size[rows, None]),
            "ea_own": np.ascontiguousarray(embed_avg[rows]),
        })
    return in_maps


def assemble(cfg: Cfg, results):
    """results: list per core of dict name->np.ndarray. Returns 6-tuple."""
    quantize = np.concatenate([results[r]["quantize_sh"] for r in range(cfg.CORES)], 0)
    embed_ind = np.concatenate([results[r]["embed_ind_sh"] for r in range(cfg.CORES)], 0)
    dist = np.concatenate([results[r]["dist_sh"] for r in range(cfg.CORES)], 0)
    embed_new = np.zeros((cfg.C, cfg.D), np.float32)
    cs_new = np.zeros((cfg.C,), np.float32)
    ea_new = np.zeros((cfg.C, cfg.D), np.float32)
    for r in range(cfg.CORES):
        rows = owned_rows(cfg, r)
        embed_new[rows] = results[r]["embed_new_sh"]
        cs_new[rows] = results[r]["cs_new_sh"][:, 0]
        ea_new[rows] = results[r]["ea_new_sh"]
    return (
        quantize[None], embed_ind[None], dist[None],
        embed_new[None], cs_new[None], ea_new[None],
    )


_CACHED = {}


def _get_nc(cfg: Cfg):
    key = (cfg.CORES, cfg.NTOK, cfg.C, cfg.D)
    if key not in _CACHED:
        _CACHED[key] = build_nc(cfg)
    return _CACHED[key]


def kernel(x, embed, cluster_size, embed_avg):
    """Full (unsharded) inputs with leading h=1 dim; returns the 6 outputs."""
    x = np.asarray(x)
    embed = np.asarray(embed)
    cluster_size = np.asarray(cluster_size)
    embed_avg = np.asarray(embed_avg)
    h, n, d = x.shape
    C = embed.shape[1]
    cfg = Cfg(cores=8, ntok=n // 8, C=C, D=d)
    nc = _get_nc(cfg)
    in_maps = make_in_maps(cfg, x[0], embed[0], cluster_size[0], embed_avg[0])
    res = bass_utils.run_bass_kernel_spmd(
        nc, in_maps, core_ids=list(range(cfg.CORES))
    )
    return assemble(cfg, res.results)


if __name__ == "__main__":
    cfg = Cfg()
    nc = build_nc(cfg)
    print("built ok:", len(nc.m.functions[0].allocations), "allocs")


# revision 13
# speedup vs baseline: 7.2260x; 7.2260x over previous
"""EuclideanCodebook (VQ) kernel for 8x TRN2 NeuronCores.

Data-parallel over tokens; embed replicated; per-chunk ReduceScatter of the
segment sums (embed_sum cols 0..D-1, bins col D) before the EMA update.

Self-contained: hardcodes problem geometry from the spec.
"""

import sys

sys.path.insert(0, "/opt/trn_rl_repo")

import numpy as np
from contextlib import ExitStack

import concourse.bass as bass
import concourse.mybir as mybir
import concourse.tile as tile
from concourse import bacc
from concourse import bass_utils
from concourse import bass_isa
from concourse.masks import make_identity

F32 = mybir.dt.float32
F32R = mybir.dt.float32r
BF16 = mybir.dt.bfloat16
I32 = mybir.dt.int32
U32 = mybir.dt.uint32
U8 = mybir.dt.uint8
P = 128

DECAY = 0.8
EPS = 1e-5


class Cfg:
    def __init__(self, cores=8, ntok=4096, C=8192, D=512):
        self.CORES = cores
        self.NTOK = ntok            # tokens per core
        self.C = C                  # codebook size
        self.D = D                  # embedding dim
        self.TILES = ntok // P      # token tiles per core
        self.DB = D // P            # d sub-blocks (4)
        self.NSEG = 4               # phase-A codebook segments resident in SBUF
        self.CSEG = C // self.NSEG
        self.CBA = min(512, self.CSEG)       # phase-A c-block (psum N)
        self.NCBA = self.CSEG // self.CBA    # c-blocks per segment
        self.NCHUNK = 8             # ReduceScatter chunks
        self.CHUNK = C // self.NCHUNK
        self.ROWS = self.CHUNK // cores      # c rows per core per chunk
        self.CBB = min(512, self.CHUNK)      # phase-B c-block (psum N)
        self.NCBB = self.CHUNK // self.CBB   # phase-B c-blocks per chunk
        self.NTOT = ntok * cores    # global token count
        assert self.ROWS <= 128 and self.CSEG % self.CBA == 0
        assert self.CHUNK % self.CBB == 0 and ntok % P == 0
        assert self.C % (self.NSEG * P) == 0


def build_nc(cfg: Cfg, mm_dt=F32R, debug=False, with_collectives=True):
    """Build the SPMD program (identical on every core)."""
    c = cfg
    SD = c.D + 1  # staged row: D embed_sum cols + 1 bins col

    nc = bacc.Bacc(
        "TRN2",
        target_bir_lowering=False,
        debug=debug,
        num_devices=c.CORES,
    )

    # ---- kernel I/O (per core) ----
    x_d = nc.dram_tensor("x_sh", [c.NTOK, c.D], F32, kind="ExternalInput")
    e_d = nc.dram_tensor("embed", [c.C, c.D], F32, kind="ExternalInput")
    csf_d = nc.dram_tensor("cs_full", [c.C // P, P], F32, kind="ExternalInput")
    cso_d = nc.dram_tensor("cs_own", [c.NCHUNK * c.ROWS, 1], F32, kind="ExternalInput")
    eao_d = nc.dram_tensor("ea_own", [c.NCHUNK * c.ROWS, c.D], F32, kind="ExternalInput")

    dist_d = nc.dram_tensor("dist_sh", [c.NTOK, c.C], F32, kind="ExternalOutput")
    ei_d = nc.dram_tensor("embed_ind_sh", [c.NTOK], I32, kind="ExternalOutput")
    q_d = nc.dram_tensor("quantize_sh", [c.NTOK, c.D], F32, kind="ExternalOutput")
    en_d = nc.dram_tensor("embed_new_sh", [c.NCHUNK * c.ROWS, c.D], F32, kind="ExternalOutput")
    csn_d = nc.dram_tensor("cs_new_sh", [c.NCHUNK * c.ROWS, 1], F32, kind="ExternalOutput")
    ean_d = nc.dram_tensor("ea_new_sh", [c.NCHUNK * c.ROWS, c.D], F32, kind="ExternalOutput")

    # ---- internal DRAM for collectives ----
    rs_in = [
        nc.dram_tensor(f"rs_in{m}", [c.CHUNK, SD], F32, kind="Internal")
        for m in range(c.NCHUNK)
    ]
    rs_out = [
        nc.dram_tensor(f"rs_out{m}", [c.ROWS, SD], F32, kind="Internal")
        for m in range(c.NCHUNK)
    ]

    groups = [list(range(c.CORES))]

    def r32(ap):
        return ap.bitcast(mm_dt)

    with ExitStack() as ctx:
        tc = ctx.enter_context(tile.TileContext(nc))

        cpool = ctx.enter_context(tc.tile_pool(name="cpool", bufs=1))
        epool = ctx.enter_context(tc.tile_pool(name="epool", bufs=3))
        etpool = ctx.enter_context(tc.tile_pool(name="etpool", bufs=1))
        xpool = ctx.enter_context(tc.tile_pool(name="xpool", bufs=2))
        wpool = ctx.enter_context(tc.tile_pool(name="wpool", bufs=2))
        dpool = ctx.enter_context(tc.tile_pool(name="dpool", bufs=2))
        mpool = ctx.enter_context(tc.tile_pool(name="mpool", bufs=1))
        bpool = ctx.enter_context(tc.tile_pool(name="bpool", bufs=3))
        sbacc = ctx.enter_context(tc.tile_pool(name="sbacc", bufs=1))
        stpool = ctx.enter_context(tc.tile_pool(name="stpool", bufs=2))
        empool = ctx.enter_context(tc.tile_pool(name="empool", bufs=2))

        spsum = ctx.enter_context(tc.tile_pool(name="spsum", bufs=2, space="PSUM"))
        tpsum = ctx.enter_context(tc.tile_pool(name="tpsum", bufs=1, space="PSUM"))
        apsum = ctx.enter_context(tc.tile_pool(name="apsum", bufs=1, space="PSUM"))

        # ================= constants =================
        ident = cpool.tile([P, P], F32)
        make_identity(nc, ident[:])

        iota_i = cpool.tile([P, c.CBB], I32)
        nc.gpsimd.iota(iota_i[:], pattern=[[1, c.CBB]], base=0, channel_multiplier=0)
        iotaf = cpool.tile([P, c.CBB], F32)
        nc.vector.tensor_copy(iotaf[:], iota_i[:])

        onn32 = cpool.tile([1, P], F32)
        nc.vector.memset(onn32[:], -0.5)
        oneneg = cpool.tile([1, P], F32R)   # lhsT for the -0.5*y2 bias matmul
        nc.scalar.copy(out=oneneg[:], in_=onn32[:])
        onecol = cpool.tile([P, 1], BF16)  # lhsT for the bins matmul
        nc.vector.memset(onecol[:], 1.0)
        junk = cpool.tile([P, c.D], F32)   # write-only sink for Square outputs

        # ================= alpha (laplace smoothing scalars) =================
        csm = cpool.tile([c.C // P, P], F32)
        nc.sync.dma_start(out=csm[:], in_=csf_d[:, :])
        csr = cpool.tile([c.C // P, 1], F32)
        nc.vector.reduce_sum(csr[:], csm[:], axis=mybir.AxisListType.X)
        csra = cpool.tile([c.C // P, 1], F32)
        nc.gpsimd.partition_all_reduce(
            csra[:], csr[:], channels=c.C // P, reduce_op=bass_isa.ReduceOp.add
        )
        total = cpool.tile([1, 1], F32)
        # total = DECAY * sum(cs) + (1-DECAY) * NTOT   (sum of bins == NTOT)
        nc.vector.tensor_scalar(
            out=total[:], in0=csra[0:1, :], scalar1=DECAY,
            scalar2=(1.0 - DECAY) * float(c.NTOT),
            op0=mybir.AluOpType.mult, op1=mybir.AluOpType.add,
        )
        denom = cpool.tile([1, 1], F32)
        nc.vector.tensor_scalar_add(denom[:], total[:], float(c.C) * EPS)
        dinv = cpool.tile([1, 1], F32)
        nc.vector.reciprocal(dinv[:], denom[:])
        alpha = cpool.tile([1, 1], F32)
        nc.vector.tensor_tensor(alpha[:], total[:], dinv[:], op=mybir.AluOpType.mult)
        acol = cpool.tile([P, 1], F32)
        nc.gpsimd.partition_broadcast(acol[:], alpha[:])
        aepscol = cpool.tile([P, 1], F32)
        nc.vector.tensor_scalar_mul(aepscol[:], acol[:], EPS)

        # x in bf16, resident for phase B
        xbf = cpool.tile([P, c.TILES * c.D], BF16)
        x2all = cpool.tile([P, c.TILES], F32)   # ||x||^2 per token

        # ================= phase A =================
        # per-segment top-2 (value + index as f32) per token, interleaved per tile
        mq = [mpool.tile([P, 2 * c.TILES], F32, name=f"mq{s}") for s in range(c.NSEG)]
        iq = [mpool.tile([P, 2 * c.TILES], F32, name=f"iq{s}") for s in range(c.NSEG)]

        for q in range(c.NSEG):
            c0 = q * c.CSEG
            # ---- build embedT (d-major) + y2 for this segment ----
            emt = etpool.tile([P, c.DB * c.CSEG], F32R, name="emt", tag="emt")
            emtv = emt[:].rearrange("p (k c) -> p k c", k=c.DB)
            y2col = etpool.tile([P, c.CSEG // P], F32, tag="y2col")
            for ct in range(c.CSEG // P):
                e_t = epool.tile([P, c.D], F32, tag="e_t")
                nc.sync.dma_start(out=e_t[:], in_=e_d[c0 + ct * P:c0 + (ct + 1) * P, :])
                nc.scalar.activation(
                    junk[:], e_t[:], mybir.ActivationFunctionType.Square,
                    accum_out=y2col[:, ct:ct + 1],
                )
                tp = tpsum.tile([P, 4 * P], F32, tag="tp")
                for k in range(c.DB):
                    nc.tensor.matmul(
                        tp[:, k * P:(k + 1) * P], lhsT=e_t[:, k * P:(k + 1) * P],
                        rhs=ident[:], is_transpose=True,
                        start=(k == 0), stop=(k == c.DB - 1),
                    )
                nc.scalar.copy(
                    out=emtv[:, :, ct * P:(ct + 1) * P],
                    in_=tp[:].rearrange("p (k c) -> p k c", k=c.DB),
                )
            # y2 row layout: (128, CSEG//128) -> transpose -> (CSEG//128, 128) -> row
            y2row = etpool.tile([1, c.CSEG], F32R, tag="y2row")
            ytp = tpsum.tile([P, 4 * P], F32, tag="tp")
            nc.tensor.transpose(ytp[:c.CSEG // P, :P], y2col[:], ident[:])
            y2tr = etpool.tile([c.CSEG // P, P], F32R, tag="y2tr")
            nc.scalar.copy(out=y2tr[:], in_=ytp[:c.CSEG // P, :P])
            nc.sync.dma_start(
                out=y2row[0:1, :].rearrange("one (a b) -> one a b", b=P),
                in_=y2tr[:],
            )

            # ---- token tiles ----
            for t in range(c.TILES):
                x_t = xpool.tile([P, c.D], F32, tag="x_t")
                nc.sync.dma_start(out=x_t[:], in_=x_d[t * P:(t + 1) * P, :])
                if q == 0:
                    nc.scalar.activation(
                        junk[:], x_t[:], mybir.ActivationFunctionType.Square,
                        accum_out=x2all[:, t:t + 1],
                    )
                    nc.vector.tensor_copy(xbf[:, t * c.D:(t + 1) * c.D], x_t[:])
                xt = xpool.tile([P, c.D], F32R, tag="xt")  # x^T: [d-part, n] blocks
                tp = tpsum.tile([P, 4 * P], F32, tag="tp")
                for k in range(c.DB):
                    nc.tensor.matmul(
                        tp[:, k * P:(k + 1) * P], lhsT=x_t[:, k * P:(k + 1) * P],
                        rhs=ident[:], is_transpose=True,
                        start=(k == 0), stop=(k == c.DB - 1),
                    )
                nc.scalar.copy(out=xt[:], in_=tp[:])

                dseg = dpool.tile([P, c.CSEG], F32, tag="dseg")
                for cb in range(c.NCBA):
                    s_ps = spsum.tile([P, c.CBA], F32, tag="s")
                    for k in range(c.DB):
                        nc.tensor.matmul(
                            s_ps[:],
                            lhsT=xt[:, k * P:(k + 1) * P],
                            rhs=emt[:, k * c.CSEG + cb * c.CBA:
                                    k * c.CSEG + (cb + 1) * c.CBA],
                            start=(k == 0), stop=False,
                        )
                    nc.tensor.matmul(
                        s_ps[:],
                        lhsT=oneneg[:],
                        rhs=y2row[0:1, cb * c.CBA:(cb + 1) * c.CBA],
                        start=False, stop=True,
                    )
                    # sq = sqrt(-2*s + x2) = ||x-e|| ; dist = -sq
                    # (the reference's clip at 0 never binds: min d2 >> 0)
                    sq_t = wpool.tile([P, c.CBA], F32, tag="sq_t")
                    nc.scalar.activation(
                        sq_t[:], s_ps[:], mybir.ActivationFunctionType.Sqrt,
                        bias=x2all[:, t:t + 1], scale=-2.0,
                    )
                    nc.gpsimd.tensor_scalar_mul(
                        dseg[:, cb * c.CBA:(cb + 1) * c.CBA], sq_t[:], -1.0
                    )
                nc.sync.dma_start(
                    out=dist_d[t * P:(t + 1) * P, c0:c0 + c.CSEG], in_=dseg[:]
                )
                m8 = wpool.tile([P, 8], F32, tag="m8")
                nc.vector.max(m8[:], dseg[:])
                i8 = wpool.tile([P, 8], U32, tag="i8")
                nc.vector.max_index(i8[:], m8[:], dseg[:])
                nc.scalar.copy(out=mq[q][:, 2 * t:2 * t + 2], in_=m8[:, 0:2])
                nc.scalar.copy(out=iq[q][:, 2 * t:2 * t + 2], in_=i8[:, 0:2])

        # ---- merge segments via exact top-2 rescore ----
        # Candidates: per-segment top-2 under f32r scores. Pick global top-2,
        # recompute their exact fp32 ||x-e||^2 and choose (ties -> lower idx).
        iqg = [mpool.tile([P, 2 * c.TILES], F32, name=f"iqg{s}") for s in range(c.NSEG)]
        for s in range(c.NSEG):
            nc.vector.tensor_scalar_add(iqg[s][:], iq[s][:], float(s * c.CSEG))
        idxf = mpool.tile([P, c.TILES], F32)
        rpool = ctx.enter_context(tc.tile_pool(name="rpool", bufs=2))
        NCAND = 2 * c.NSEG
        for t in range(c.TILES):
            cm = rpool.tile([P, NCAND], F32, tag="cm")
            ci = rpool.tile([P, NCAND], F32, tag="ci")
            for s in range(c.NSEG):
                nc.vector.tensor_copy(cm[:, 2 * s:2 * s + 2], mq[s][:, 2 * t:2 * t + 2])
                nc.vector.tensor_copy(ci[:, 2 * s:2 * s + 2], iqg[s][:, 2 * t:2 * t + 2])
            s8 = rpool.tile([P, 8], F32, tag="s8")
            nc.vector.max(s8[:], cm[:])
            p8 = rpool.tile([P, 8], U32, tag="p8")
            nc.vector.max_index(p8[:], s8[:], cm[:])
            p8f = rpool.tile([P, 8], F32, tag="p8f")
            nc.vector.tensor_copy(p8f[:], p8[:])
            xr = rpool.tile([P, c.D], F32, tag="xr")
            nc.sync.dma_start(out=xr[:], in_=x_d[t * P:(t + 1) * P, :])
            iand = []
            d2 = []
            for j in range(2):
                sel = rpool.tile([P, NCAND], F32, tag=f"sel{j}")
                nc.vector.scalar_tensor_tensor(
                    out=sel[:], in0=iotaf[:, :NCAND], scalar=p8f[:, j:j + 1],
                    in1=ci[:], op0=mybir.AluOpType.is_equal,
                    op1=mybir.AluOpType.mult,
                )
                icf = rpool.tile([P, 1], F32, tag=f"icf{j}")
                nc.vector.reduce_sum(icf[:], sel[:], axis=mybir.AxisListType.X)
                ici = rpool.tile([P, 1], I32, tag=f"ici{j}")
                nc.vector.tensor_copy(ici[:], icf[:])
                iand.append((icf, ici))
                g_j = rpool.tile([P, c.D], F32, tag=f"g{j}")
                nc.gpsimd.indirect_dma_start(
                    out=g_j[:], out_offset=None, in_=e_d[:],
                    in_offset=bass.IndirectOffsetOnAxis(ap=ici[:, 0:1], axis=0),
                )
                df = rpool.tile([P, c.D], F32, tag=f"df{j}")
                nc.vector.tensor_tensor(df[:], xr[:], g_j[:],
                                        op=mybir.AluOpType.subtract)
                nc.vector.tensor_tensor(df[:], df[:], df[:],
                                        op=mybir.AluOpType.mult)
                rr = rpool.tile([P, c.DB], F32, tag=f"rr{j}")
                nc.vector.reduce_sum(
                    rr[:], df[:].rearrange("p (a b) -> p a b", b=P),
                    axis=mybir.AxisListType.X,
                )
                d2j = rpool.tile([P, 1], F32, tag=f"d2{j}")
                nc.vector.reduce_sum(d2j[:], rr[:], axis=mybir.AxisListType.X)
                d2.append(d2j)
            lt = rpool.tile([P, 1], U8, tag="lt")
            nc.vector.tensor_tensor(lt[:], d2[0][:], d2[1][:],
                                    op=mybir.AluOpType.is_lt)
            eqm = rpool.tile([P, 1], U8, tag="eqm")
            nc.vector.tensor_tensor(eqm[:], d2[0][:], d2[1][:],
                                    op=mybir.AluOpType.is_equal)
            imin = rpool.tile([P, 1], F32, tag="imin")
            nc.vector.tensor_tensor(imin[:], iand[0][0][:], iand[1][0][:],
                                    op=mybir.AluOpType.min)
            tmpi = rpool.tile([P, 1], F32, tag="tmpi")
            nc.vector.select(tmpi[:], mask=lt[:], on_true=iand[0][0][:],
                             on_false=iand[1][0][:])
            nc.vector.select(idxf[:, t:t + 1], mask=eqm[:], on_true=imin[:],
                             on_false=tmpi[:])
        idxi = mpool.tile([P, c.TILES], I32)
        nc.vector.tensor_copy(idxi[:], idxf[:])

        # ---- embed_ind output ----
        eit = tpsum.tile([P, P], F32, tag="tp")
        nc.tensor.transpose(eit[:c.TILES, :], idxf[:], ident[:])
        eisb = mpool.tile([c.TILES, P], I32)
        nc.vector.tensor_copy(eisb[:], eit[:c.TILES, :])
        nc.sync.dma_start(
            out=ei_d[:].rearrange("(t p) -> t p", p=P), in_=eisb[:]
        )

        # ---- quantize: gather embed rows ----
        for t in range(c.TILES):
            q_t = bpool.tile([P, c.D], F32, tag="q_t")
            nc.gpsimd.indirect_dma_start(
                out=q_t[:],
                out_offset=None,
                in_=e_d[:],
                in_offset=bass.IndirectOffsetOnAxis(ap=idxi[:, t:t + 1], axis=0),
            )
            nc.sync.dma_start(out=q_d[t * P:(t + 1) * P, :], in_=q_t[:])

        # ================= phase B: segment sums + ReduceScatter =================
        for m in range(c.NCHUNK):
            for cbl in range(c.NCBB):
                cb0 = m * c.CHUNK + cbl * c.CBB  # global c offset of this block
                idsh = bpool.tile([P, c.TILES], F32, tag="idsh")
                nc.vector.tensor_scalar_add(idsh[:], idxf[:], -float(cb0))
                acc = [
                    apsum.tile([P, c.CBB], F32, name=f"acc{k}", tag=f"acc{k}")
                    for k in range(c.DB)
                ]
                accb = apsum.tile([1, c.CBB], F32, tag="accb")
                for t in range(c.TILES):
                    oh = bpool.tile([P, c.CBB], BF16, tag="oh")
                    nc.gpsimd.tensor_scalar(
                        out=oh[:], in0=iotaf[:, :c.CBB], scalar1=idsh[:, t:t + 1],
                        scalar2=None, op0=mybir.AluOpType.is_equal,
                    )
                    for k in range(c.DB):
                        nc.tensor.matmul(
                            acc[k][:],
                            lhsT=xbf[:, t * c.D + k * P:t * c.D + (k + 1) * P],
                            rhs=oh[:],
                            start=(t == 0), stop=(t == c.TILES - 1),
                        )
                    nc.tensor.matmul(
                        accb[:],
                        lhsT=onecol[:],
                        rhs=oh[:],
                        start=(t == 0), stop=(t == c.TILES - 1),
                    )
                # copy psum accumulators to sbuf
                asb = [
                    sbacc.tile([P, c.CBB], F32, name=f"asb{k}", tag=f"asb{k}")
                    for k in range(c.DB)
                ]
                absb = sbacc.tile([1, c.CBB], F32, tag="absb")
                for k in range(c.DB):
                    nc.scalar.copy(out=asb[k][:], in_=acc[k][:])
                nc.vector.tensor_copy(absb[:], accb[:])
                # transpose to c-major and stage
                for j in range(c.CBB // P):
                    stg = stpool.tile([P, SD], F32, tag="stg")
                    tp = tpsum.tile([P, 4 * P], F32, tag="tp")
                    for k in range(c.DB):
                        nc.tensor.matmul(
                            tp[:, k * P:(k + 1) * P],
                            lhsT=asb[k][:, j * P:(j + 1) * P],
                            rhs=ident[:], is_transpose=True,
                            start=(k == 0), stop=(k == c.DB - 1),
                        )
                    nc.scalar.copy(out=stg[:, :c.DB * P], in_=tp[:])
                    tpb = spsum.tile([P, c.CBA], F32, tag="s")
                    nc.tensor.transpose(
                        tpb[:, 0:1], absb[0:1, j * P:(j + 1) * P], ident[0:1, 0:1]
                    )
                    nc.scalar.copy(out=stg[:, c.D:SD], in_=tpb[:, 0:1])
                    nc.sync.dma_start(
                        out=rs_in[m][cbl * c.CBB + j * P:cbl * c.CBB + (j + 1) * P, :],
                        in_=stg[:],
                    )
            if with_collectives:
                nc.gpsimd.collective_compute(
                    "ReduceScatter",
                    mybir.AluOpType.add,
                    replica_groups=groups,
                    ins=[rs_in[m].ap().opt()],
                    outs=[rs_out[m].ap().opt()],
                )
            else:
                # timing-model stub: keeps the dataflow shape without comms
                nc.sync.dma_start(out=rs_out[m][:, :], in_=rs_in[m][0:c.ROWS, :])

        # ================= EMA update on owned c rows =================
        for m in range(c.NCHUNK):
            es = empool.tile([c.ROWS, SD], F32, tag="es")
            nc.sync.dma_start(out=es[:], in_=rs_out[m][:, :])
            ea = empool.tile([c.ROWS, c.D], F32, tag="ea")
            nc.sync.dma_start(out=ea[:], in_=eao_d[m * c.ROWS:(m + 1) * c.ROWS, :])
            csc = empool.tile([c.ROWS, 1], F32, tag="csc")
            nc.sync.dma_start(out=csc[:], in_=cso_d[m * c.ROWS:(m + 1) * c.ROWS, :])

            cs08 = empool.tile([c.ROWS, 1], F32, tag="cs08")
            nc.vector.tensor_scalar_mul(cs08[:], csc[:], DECAY)
            csn = empool.tile([c.ROWS, 1], F32, tag="csn")
            nc.vector.scalar_tensor_tensor(
                out=csn[:], in0=es[:, c.D:SD], scalar=1.0 - DECAY, in1=cs08[:],
                op0=mybir.AluOpType.mult, op1=mybir.AluOpType.add,
            )
            ea08 = empool.tile([c.ROWS, c.D], F32, tag="ea08")
            nc.vector.tensor_scalar_mul(ea08[:], ea[:], DECAY)
            ean = empool.tile([c.ROWS, c.D], F32, tag="ean")
            nc.vector.scalar_tensor_tensor(
                out=ean[:], in0=es[:, :c.D], scalar=1.0 - DECAY, in1=ea08[:],
                op0=mybir.AluOpType.mult, op1=mybir.AluOpType.add,
            )
            smo = empool.tile([c.ROWS, 1], F32, tag="smo")
            nc.vector.scalar_tensor_tensor(
                out=smo[:], in0=csn[:], scalar=acol[:c.ROWS, :], in1=aepscol[:c.ROWS, :],
                op0=mybir.AluOpType.mult, op1=mybir.AluOpType.add,
            )
            sinv = empool.tile([c.ROWS, 1], F32, tag="sinv")
            nc.vector.reciprocal(sinv[:], smo[:])
            en = empool.tile([c.ROWS, c.D], F32, tag="en")
            nc.vector.tensor_scalar_mul(en[:], ean[:], sinv[:])

            nc.sync.dma_start(out=en_d[m * c.ROWS:(m + 1) * c.ROWS, :], in_=en[:])
            nc.sync.dma_start(out=csn_d[m * c.ROWS:(m + 1) * c.ROWS, :], in_=csn[:])
            nc.sync.dma_start(out=ean_d[m * c.ROWS:(m + 1) * c.ROWS, :], in_=ean[:])

    nc.compile()
    return nc


# ======================= host-side glue =======================

def owned_rows(cfg: Cfg, r: int) -> np.ndarray:
    """Global c indices owned by core r (concatenated per RS chunk)."""
    return np.concatenate([
        np.arange(m * cfg.CHUNK + r * cfg.ROWS, m * cfg.CHUNK + (r + 1) * cfg.ROWS)
        for m in range(cfg.NCHUNK)
    ])


def make_in_maps(cfg: Cfg, x, embed, cluster_size, embed_avg):
    """x: (n, d) full; embed: (C, D); cluster_size: (C,); embed_avg: (C, D)."""
    in_maps = []
    for r in range(cfg.CORES):
        rows = owned_rows(cfg, r)
        in_maps.append({
            "x_sh": np.ascontiguousarray(x[r * cfg.NTOK:(r + 1) * cfg.NTOK]),
            "embed": np.ascontiguousarray(embed),
            "cs_full": np.ascontiguousarray(cluster_size.reshape(cfg.C // P, P)),
            "cs_own": np.ascontiguousarray(cluster_size[rows, None]),
            "ea_own": np.ascontiguousarray(embed_avg[rows]),
        })
    return in_maps


def assemble(cfg: Cfg, results):
    """results: list per core of dict name->np.ndarray. Returns 6-tuple."""
    quantize = np.concatenate([results[r]["quantize_sh"] for r in range(cfg.CORES)], 0)
    embed_ind = np.concatenate([results[r]["embed_ind_sh"] for r in range(cfg.CORES)], 0)
    dist = np.concatenate([results[r]["dist_sh"] for r in range(cfg.CORES)], 0)
    embed_new = np.zeros((cfg.C, cfg.D), np.float32)
    cs_new = np.zeros((cfg.C,), np.float32)
    ea_new = np.zeros((cfg.C, cfg.D), np.float32)
    for r in range(cfg.CORES):
        rows = owned_rows(cfg, r)
        embed_new[rows] = results[r]["embed_new_sh"]
        cs_new[rows] = results[r]["cs_new_sh"][:, 0]
        ea_new[rows] = results[r]["ea_new_sh"]
    return (
        quantize[None], embed_ind[None], dist[None],
        embed_new[None], cs_new[None], ea_new[None],
    )


_CACHED = {}


def _get_nc(cfg: Cfg):
    key = (cfg.CORES, cfg.NTOK, cfg.C, cfg.D)
    if key not in _CACHED:
        _CACHED[key] = build_nc(cfg)
    return _CACHED[key]


def kernel(x, embed, cluster_size, embed_avg):
    """Full (unsharded) inputs with leading h=1 dim; returns the 6 outputs."""
    x = np.asarray(x)
    embed = np.asarray(embed)
    cluster_size = np.asarray(cluster_size)
    embed_avg = np.asarray(embed_avg)
    h, n, d = x.shape
    C = embed.shape[1]
    cfg = Cfg(cores=8, ntok=n // 8, C=C, D=d)
    nc = _get_nc(cfg)
    in_maps = make_in_maps(cfg, x[0], embed[0], cluster_size[0], embed_avg[0])
    res = bass_utils.run_bass_kernel_spmd(
        nc, in_maps, core_ids=list(range(cfg.CORES))
    )
    return assemble(cfg, res.results)


if __name__ == "__main__":
    cfg = Cfg()
    nc = build_nc(cfg)
    print("built ok:", len(nc.m.functions[0].allocations), "allocs")


# revision 14
# speedup vs baseline: 13.8786x; 1.9206x over previous
"""EuclideanCodebook (VQ) kernel for 8x TRN2 NeuronCores.

Data-parallel over tokens; embed replicated; per-chunk ReduceScatter of the
segment sums (embed_sum cols 0..D-1, bins col D) before the EMA update.

Self-contained: hardcodes problem geometry from the spec.
"""

import sys

sys.path.insert(0, "/opt/trn_rl_repo")

import numpy as np
from contextlib import ExitStack

import concourse.bass as bass
import concourse.mybir as mybir
import concourse.tile as tile
from concourse import bacc
from concourse import bass_utils
from concourse import bass_isa
from concourse.masks import make_identity

F32 = mybir.dt.float32
F32R = mybir.dt.float32r
BF16 = mybir.dt.bfloat16
I32 = mybir.dt.int32
U32 = mybir.dt.uint32
U8 = mybir.dt.uint8
P = 128

DECAY = 0.8
EPS = 1e-5


class Cfg:
    def __init__(self, cores=8, ntok=4096, C=8192, D=512):
        self.CORES = cores
        self.NTOK = ntok            # tokens per core
        self.C = C                  # codebook size
        self.D = D                  # embedding dim
        self.TILES = ntok // P      # token tiles per core
        self.DB = D // P            # d sub-blocks (4)
        self.NSEG = 4               # phase-A codebook segments resident in SBUF
        self.CSEG = C // self.NSEG
        self.CBA = min(512, self.CSEG)       # phase-A c-block (psum N)
        self.NCBA = self.CSEG // self.CBA    # c-blocks per segment
        self.NCHUNK = 8             # ReduceScatter chunks
        self.CHUNK = C // self.NCHUNK
        self.ROWS = self.CHUNK // cores      # c rows per core per chunk
        self.CBB = min(512, self.CHUNK)      # phase-B c-block (psum N)
        self.NCBB = self.CHUNK // self.CBB   # phase-B c-blocks per chunk
        self.NTOT = ntok * cores    # global token count
        assert self.ROWS <= 128 and self.CSEG % self.CBA == 0
        assert self.CHUNK % self.CBB == 0 and ntok % P == 0
        assert self.C % (self.NSEG * P) == 0


def build_nc(cfg: Cfg, mm_dt=F32R, debug=False, with_collectives=True,
             neg_engine="gpsimd", oh_engine="gpsimd"):
    """Build the SPMD program (identical on every core)."""
    c = cfg
    SD = c.D + 1  # staged row: D embed_sum cols + 1 bins col

    nc = bacc.Bacc(
        "TRN2",
        target_bir_lowering=False,
        debug=debug,
        num_devices=c.CORES,
    )

    # ---- kernel I/O (per core) ----
    x_d = nc.dram_tensor("x_sh", [c.NTOK, c.D], F32, kind="ExternalInput")
    e_d = nc.dram_tensor("embed", [c.C, c.D], F32, kind="ExternalInput")
    csf_d = nc.dram_tensor("cs_full", [c.C // P, P], F32, kind="ExternalInput")
    cso_d = nc.dram_tensor("cs_own", [c.NCHUNK * c.ROWS, 1], F32, kind="ExternalInput")
    eao_d = nc.dram_tensor("ea_own", [c.NCHUNK * c.ROWS, c.D], F32, kind="ExternalInput")

    dist_d = nc.dram_tensor("dist_sh", [c.NTOK, c.C], F32, kind="ExternalOutput")
    ei_d = nc.dram_tensor("embed_ind_sh", [c.NTOK], I32, kind="ExternalOutput")
    q_d = nc.dram_tensor("quantize_sh", [c.NTOK, c.D], F32, kind="ExternalOutput")
    en_d = nc.dram_tensor("embed_new_sh", [c.NCHUNK * c.ROWS, c.D], F32, kind="ExternalOutput")
    csn_d = nc.dram_tensor("cs_new_sh", [c.NCHUNK * c.ROWS, 1], F32, kind="ExternalOutput")
    ean_d = nc.dram_tensor("ea_new_sh", [c.NCHUNK * c.ROWS, c.D], F32, kind="ExternalOutput")

    # ---- internal DRAM for collectives ----
    rs_in = [
        nc.dram_tensor(f"rs_in{m}", [c.CHUNK, SD], F32, kind="Internal")
        for m in range(c.NCHUNK)
    ]
    rs_out = [
        nc.dram_tensor(f"rs_out{m}", [c.ROWS, SD], F32, kind="Internal")
        for m in range(c.NCHUNK)
    ]

    groups = [list(range(c.CORES))]

    def r32(ap):
        return ap.bitcast(mm_dt)

    with ExitStack() as ctx:
        tc = ctx.enter_context(tile.TileContext(nc))

        cpool = ctx.enter_context(tc.tile_pool(name="cpool", bufs=1))
        epool = ctx.enter_context(tc.tile_pool(name="epool", bufs=3))
        etpool = ctx.enter_context(tc.tile_pool(name="etpool", bufs=1))
        xpool = ctx.enter_context(tc.tile_pool(name="xpool", bufs=2))
        wpool = ctx.enter_context(tc.tile_pool(name="wpool", bufs=2))
        dpool = ctx.enter_context(tc.tile_pool(name="dpool", bufs=2))
        mpool = ctx.enter_context(tc.tile_pool(name="mpool", bufs=1))
        bpool = ctx.enter_context(tc.tile_pool(name="bpool", bufs=3))
        sbacc = ctx.enter_context(tc.tile_pool(name="sbacc", bufs=1))
        stpool = ctx.enter_context(tc.tile_pool(name="stpool", bufs=2))
        empool = ctx.enter_context(tc.tile_pool(name="empool", bufs=2))

        spsum = ctx.enter_context(tc.tile_pool(name="spsum", bufs=2, space="PSUM"))
        tpsum = ctx.enter_context(tc.tile_pool(name="tpsum", bufs=1, space="PSUM"))
        apsum = ctx.enter_context(tc.tile_pool(name="apsum", bufs=1, space="PSUM"))

        # ================= constants =================
        ident = cpool.tile([P, P], F32)
        make_identity(nc, ident[:])

        iota_i = cpool.tile([P, c.CBB], I32)
        nc.gpsimd.iota(iota_i[:], pattern=[[1, c.CBB]], base=0, channel_multiplier=0)
        iotaf = cpool.tile([P, c.CBB], F32)
        nc.vector.tensor_copy(iotaf[:], iota_i[:])

        onn32 = cpool.tile([1, P], F32)
        nc.vector.memset(onn32[:], -0.5)
        oneneg = cpool.tile([1, P], F32R)   # lhsT for the -0.5*y2 bias matmul
        nc.scalar.copy(out=oneneg[:], in_=onn32[:])
        onecol = cpool.tile([P, 1], BF16)  # lhsT for the bins matmul
        nc.vector.memset(onecol[:], 1.0)
        junk = cpool.tile([P, c.D], F32)   # write-only sink for Square outputs

        # ================= alpha (laplace smoothing scalars) =================
        csm = cpool.tile([c.C // P, P], F32)
        nc.sync.dma_start(out=csm[:], in_=csf_d[:, :])
        csr = cpool.tile([c.C // P, 1], F32)
        nc.vector.reduce_sum(csr[:], csm[:], axis=mybir.AxisListType.X)
        csra = cpool.tile([c.C // P, 1], F32)
        nc.gpsimd.partition_all_reduce(
            csra[:], csr[:], channels=c.C // P, reduce_op=bass_isa.ReduceOp.add
        )
        total = cpool.tile([1, 1], F32)
        # total = DECAY * sum(cs) + (1-DECAY) * NTOT   (sum of bins == NTOT)
        nc.vector.tensor_scalar(
            out=total[:], in0=csra[0:1, :], scalar1=DECAY,
            scalar2=(1.0 - DECAY) * float(c.NTOT),
            op0=mybir.AluOpType.mult, op1=mybir.AluOpType.add,
        )
        denom = cpool.tile([1, 1], F32)
        nc.vector.tensor_scalar_add(denom[:], total[:], float(c.C) * EPS)
        dinv = cpool.tile([1, 1], F32)
        nc.vector.reciprocal(dinv[:], denom[:])
        alpha = cpool.tile([1, 1], F32)
        nc.vector.tensor_tensor(alpha[:], total[:], dinv[:], op=mybir.AluOpType.mult)
        acol = cpool.tile([P, 1], F32)
        nc.gpsimd.partition_broadcast(acol[:], alpha[:])
        aepscol = cpool.tile([P, 1], F32)
        nc.vector.tensor_scalar_mul(aepscol[:], acol[:], EPS)

        # x in bf16, resident for phase B
        xbf = cpool.tile([P, c.TILES * c.D], BF16)
        x2all = cpool.tile([P, c.TILES], F32)   # ||x||^2 per token

        # ================= phase A =================
        # per-segment top-2 (value + index as f32) per token, interleaved per tile
        mq = [mpool.tile([P, 2 * c.TILES], F32, name=f"mq{s}") for s in range(c.NSEG)]
        iq = [mpool.tile([P, 2 * c.TILES], F32, name=f"iq{s}") for s in range(c.NSEG)]

        for q in range(c.NSEG):
            c0 = q * c.CSEG
            # ---- build embedT (d-major) + y2 for this segment ----
            emt = etpool.tile([P, c.DB * c.CSEG], F32R, name="emt", tag="emt")
            emtv = emt[:].rearrange("p (k c) -> p k c", k=c.DB)
            y2col = etpool.tile([P, c.CSEG // P], F32, tag="y2col")
            for ct in range(c.CSEG // P):
                e_t = epool.tile([P, c.D], F32, tag="e_t")
                nc.sync.dma_start(out=e_t[:], in_=e_d[c0 + ct * P:c0 + (ct + 1) * P, :])
                nc.scalar.activation(
                    junk[:], e_t[:], mybir.ActivationFunctionType.Square,
                    accum_out=y2col[:, ct:ct + 1],
                )
                tp = tpsum.tile([P, 4 * P], F32, tag="tp")
                for k in range(c.DB):
                    nc.tensor.matmul(
                        tp[:, k * P:(k + 1) * P], lhsT=e_t[:, k * P:(k + 1) * P],
                        rhs=ident[:], is_transpose=True,
                        start=(k == 0), stop=(k == c.DB - 1),
                    )
                nc.scalar.copy(
                    out=emtv[:, :, ct * P:(ct + 1) * P],
                    in_=tp[:].rearrange("p (k c) -> p k c", k=c.DB),
                )
            # y2 row layout: (128, CSEG//128) -> transpose -> (CSEG//128, 128) -> row
            y2row = etpool.tile([1, c.CSEG], F32R, tag="y2row")
            ytp = tpsum.tile([P, 4 * P], F32, tag="tp")
            nc.tensor.transpose(ytp[:c.CSEG // P, :P], y2col[:], ident[:])
            y2tr = etpool.tile([c.CSEG // P, P], F32R, tag="y2tr")
            nc.scalar.copy(out=y2tr[:], in_=ytp[:c.CSEG // P, :P])
            nc.sync.dma_start(
                out=y2row[0:1, :].rearrange("one (a b) -> one a b", b=P),
                in_=y2tr[:],
            )

            # ---- token tiles ----
            for t in range(c.TILES):
                x_t = xpool.tile([P, c.D], F32, tag="x_t")
                nc.sync.dma_start(out=x_t[:], in_=x_d[t * P:(t + 1) * P, :])
                if q == 0:
                    nc.scalar.activation(
                        junk[:], x_t[:], mybir.ActivationFunctionType.Square,
                        accum_out=x2all[:, t:t + 1],
                    )
                    nc.vector.tensor_copy(xbf[:, t * c.D:(t + 1) * c.D], x_t[:])
                xt = xpool.tile([P, c.D], F32R, tag="xt")  # x^T: [d-part, n] blocks
                tp = tpsum.tile([P, 4 * P], F32, tag="tp")
                for k in range(c.DB):
                    nc.tensor.matmul(
                        tp[:, k * P:(k + 1) * P], lhsT=x_t[:, k * P:(k + 1) * P],
                        rhs=ident[:], is_transpose=True,
                        start=(k == 0), stop=(k == c.DB - 1),
                    )
                nc.scalar.copy(out=xt[:], in_=tp[:])

                dseg = dpool.tile([P, c.CSEG], F32, tag="dseg")
                for cb in range(c.NCBA):
                    s_ps = spsum.tile([P, c.CBA], F32, tag="s")
                    for k in range(c.DB):
                        nc.tensor.matmul(
                            s_ps[:],
                            lhsT=xt[:, k * P:(k + 1) * P],
                            rhs=emt[:, k * c.CSEG + cb * c.CBA:
                                    k * c.CSEG + (cb + 1) * c.CBA],
                            start=(k == 0), stop=False,
                        )
                    nc.tensor.matmul(
                        s_ps[:],
                        lhsT=oneneg[:],
                        rhs=y2row[0:1, cb * c.CBA:(cb + 1) * c.CBA],
                        start=False, stop=True,
                    )
                    # sq = sqrt(-2*s + x2) = ||x-e|| ; dist = -sq
                    # (the reference's clip at 0 never binds: min d2 >> 0)
                    sq_t = wpool.tile([P, c.CBA], F32, tag="sq_t")
                    nc.scalar.activation(
                        sq_t[:], s_ps[:], mybir.ActivationFunctionType.Sqrt,
                        bias=x2all[:, t:t + 1], scale=-2.0,
                    )
                    if neg_engine == "scalar":
                        nc.scalar.mul(
                            out=dseg[:, cb * c.CBA:(cb + 1) * c.CBA],
                            in_=sq_t[:], mul=-1.0,
                        )
                    else:
                        getattr(nc, neg_engine).tensor_scalar_mul(
                            dseg[:, cb * c.CBA:(cb + 1) * c.CBA], sq_t[:], -1.0
                        )
                nc.sync.dma_start(
                    out=dist_d[t * P:(t + 1) * P, c0:c0 + c.CSEG], in_=dseg[:]
                )
                m8 = wpool.tile([P, 8], F32, tag="m8")
                nc.vector.max(m8[:], dseg[:])
                i8 = wpool.tile([P, 8], U32, tag="i8")
                nc.vector.max_index(i8[:], m8[:], dseg[:])
                nc.scalar.copy(out=mq[q][:, 2 * t:2 * t + 2], in_=m8[:, 0:2])
                nc.scalar.copy(out=iq[q][:, 2 * t:2 * t + 2], in_=i8[:, 0:2])

        # ---- merge segments via exact top-2 rescore ----
        # Candidates: per-segment top-2 under f32r scores. Pick global top-2,
        # recompute their exact fp32 ||x-e||^2 and choose (ties -> lower idx).
        iqg = [mpool.tile([P, 2 * c.TILES], F32, name=f"iqg{s}") for s in range(c.NSEG)]
        for s in range(c.NSEG):
            nc.vector.tensor_scalar_add(iqg[s][:], iq[s][:], float(s * c.CSEG))
        idxf = mpool.tile([P, c.TILES], F32)
        rpool = ctx.enter_context(tc.tile_pool(name="rpool", bufs=2))
        NCAND = 2 * c.NSEG
        for t in range(c.TILES):
            cm = rpool.tile([P, NCAND], F32, tag="cm")
            ci = rpool.tile([P, NCAND], F32, tag="ci")
            for s in range(c.NSEG):
                nc.vector.tensor_copy(cm[:, 2 * s:2 * s + 2], mq[s][:, 2 * t:2 * t + 2])
                nc.vector.tensor_copy(ci[:, 2 * s:2 * s + 2], iqg[s][:, 2 * t:2 * t + 2])
            s8 = rpool.tile([P, 8], F32, tag="s8")
            nc.vector.max(s8[:], cm[:])
            p8 = rpool.tile([P, 8], U32, tag="p8")
            nc.vector.max_index(p8[:], s8[:], cm[:])
            p8f = rpool.tile([P, 8], F32, tag="p8f")
            nc.vector.tensor_copy(p8f[:], p8[:])
            xr = rpool.tile([P, c.D], F32, tag="xr")
            nc.sync.dma_start(out=xr[:], in_=x_d[t * P:(t + 1) * P, :])
            iand = []
            d2 = []
            for j in range(2):
                sel = rpool.tile([P, NCAND], F32, tag=f"sel{j}")
                nc.vector.scalar_tensor_tensor(
                    out=sel[:], in0=iotaf[:, :NCAND], scalar=p8f[:, j:j + 1],
                    in1=ci[:], op0=mybir.AluOpType.is_equal,
                    op1=mybir.AluOpType.mult,
                )
                icf = rpool.tile([P, 1], F32, tag=f"icf{j}")
                nc.vector.reduce_sum(icf[:], sel[:], axis=mybir.AxisListType.X)
                ici = rpool.tile([P, 1], I32, tag=f"ici{j}")
                nc.vector.tensor_copy(ici[:], icf[:])
                iand.append((icf, ici))
                g_j = rpool.tile([P, c.D], F32, tag=f"g{j}")
                nc.gpsimd.indirect_dma_start(
                    out=g_j[:], out_offset=None, in_=e_d[:],
                    in_offset=bass.IndirectOffsetOnAxis(ap=ici[:, 0:1], axis=0),
                )
                df = rpool.tile([P, c.D], F32, tag=f"df{j}")
                nc.vector.tensor_tensor(df[:], xr[:], g_j[:],
                                        op=mybir.AluOpType.subtract)
                nc.vector.tensor_tensor(df[:], df[:], df[:],
                                        op=mybir.AluOpType.mult)
                rr = rpool.tile([P, c.DB], F32, tag=f"rr{j}")
                nc.vector.reduce_sum(
                    rr[:], df[:].rearrange("p (a b) -> p a b", b=P),
                    axis=mybir.AxisListType.X,
                )
                d2j = rpool.tile([P, 1], F32, tag=f"d2{j}")
                nc.vector.reduce_sum(d2j[:], rr[:], axis=mybir.AxisListType.X)
                d2.append(d2j)
            lt = rpool.tile([P, 1], U8, tag="lt")
            nc.vector.tensor_tensor(lt[:], d2[0][:], d2[1][:],
                                    op=mybir.AluOpType.is_lt)
            eqm = rpool.tile([P, 1], U8, tag="eqm")
            nc.vector.tensor_tensor(eqm[:], d2[0][:], d2[1][:],
                                    op=mybir.AluOpType.is_equal)
            imin = rpool.tile([P, 1], F32, tag="imin")
            nc.vector.tensor_tensor(imin[:], iand[0][0][:], iand[1][0][:],
                                    op=mybir.AluOpType.min)
            tmpi = rpool.tile([P, 1], F32, tag="tmpi")
            nc.vector.select(tmpi[:], mask=lt[:], on_true=iand[0][0][:],
                             on_false=iand[1][0][:])
            nc.vector.select(idxf[:, t:t + 1], mask=eqm[:], on_true=imin[:],
                             on_false=tmpi[:])
        idxi = mpool.tile([P, c.TILES], I32)
        nc.vector.tensor_copy(idxi[:], idxf[:])

        # ---- embed_ind output ----
        eit = tpsum.tile([P, P], F32, tag="tp")
        nc.tensor.transpose(eit[:c.TILES, :], idxf[:], ident[:])
        eisb = mpool.tile([c.TILES, P], I32)
        nc.vector.tensor_copy(eisb[:], eit[:c.TILES, :])
        nc.sync.dma_start(
            out=ei_d[:].rearrange("(t p) -> t p", p=P), in_=eisb[:]
        )

        # ---- quantize: gather embed rows ----
        for t in range(c.TILES):
            q_t = bpool.tile([P, c.D], F32, tag="q_t")
            nc.gpsimd.indirect_dma_start(
                out=q_t[:],
                out_offset=None,
                in_=e_d[:],
                in_offset=bass.IndirectOffsetOnAxis(ap=idxi[:, t:t + 1], axis=0),
            )
            nc.sync.dma_start(out=q_d[t * P:(t + 1) * P, :], in_=q_t[:])

        # ================= phase B: segment sums + ReduceScatter =================
        for m in range(c.NCHUNK):
            for cbl in range(c.NCBB):
                cb0 = m * c.CHUNK + cbl * c.CBB  # global c offset of this block
                idsh = bpool.tile([P, c.TILES], F32, tag="idsh")
                nc.vector.tensor_scalar_add(idsh[:], idxf[:], -float(cb0))
                acc = [
                    apsum.tile([P, c.CBB], F32, name=f"acc{k}", tag=f"acc{k}")
                    for k in range(c.DB)
                ]
                accb = apsum.tile([1, c.CBB], F32, tag="accb")
                for t in range(c.TILES):
                    oh = bpool.tile([P, c.CBB], BF16, tag="oh")
                    getattr(nc, oh_engine).tensor_scalar(
                        out=oh[:], in0=iotaf[:, :c.CBB], scalar1=idsh[:, t:t + 1],
                        scalar2=None, op0=mybir.AluOpType.is_equal,
                    )
                    for k in range(c.DB):
                        nc.tensor.matmul(
                            acc[k][:],
                            lhsT=xbf[:, t * c.D + k * P:t * c.D + (k + 1) * P],
                            rhs=oh[:],
                            start=(t == 0), stop=(t == c.TILES - 1),
                        )
                    nc.tensor.matmul(
                        accb[:],
                        lhsT=onecol[:],
                        rhs=oh[:],
                        start=(t == 0), stop=(t == c.TILES - 1),
                    )
                # copy psum accumulators to sbuf
                asb = [
                    sbacc.tile([P, c.CBB], F32, name=f"asb{k}", tag=f"asb{k}")
                    for k in range(c.DB)
                ]
                absb = sbacc.tile([1, c.CBB], F32, tag="absb")
                for k in range(c.DB):
                    nc.scalar.copy(out=asb[k][:], in_=acc[k][:])
                nc.vector.tensor_copy(absb[:], accb[:])
                # transpose to c-major and stage
                for j in range(c.CBB // P):
                    stg = stpool.tile([P, SD], F32, tag="stg")
                    tp = tpsum.tile([P, 4 * P], F32, tag="tp")
                    for k in range(c.DB):
                        nc.tensor.matmul(
                            tp[:, k * P:(k + 1) * P],
                            lhsT=asb[k][:, j * P:(j + 1) * P],
                            rhs=ident[:], is_transpose=True,
                            start=(k == 0), stop=(k == c.DB - 1),
                        )
                    nc.scalar.copy(out=stg[:, :c.DB * P], in_=tp[:])
                    tpb = spsum.tile([P, c.CBA], F32, tag="s")
                    nc.tensor.transpose(
                        tpb[:, 0:1], absb[0:1, j * P:(j + 1) * P], ident[0:1, 0:1]
                    )
                    nc.scalar.copy(out=stg[:, c.D:SD], in_=tpb[:, 0:1])
                    nc.sync.dma_start(
                        out=rs_in[m][cbl * c.CBB + j * P:cbl * c.CBB + (j + 1) * P, :],
                        in_=stg[:],
                    )
            if with_collectives:
                nc.gpsimd.collective_compute(
                    "ReduceScatter",
                    mybir.AluOpType.add,
                    replica_groups=groups,
                    ins=[rs_in[m].ap().opt()],
                    outs=[rs_out[m].ap().opt()],
                )
            else:
                # timing-model stub: keeps the dataflow shape without comms
                nc.sync.dma_start(out=rs_out[m][:, :], in_=rs_in[m][0:c.ROWS, :])

        # ================= EMA update on owned c rows =================
        for m in range(c.NCHUNK):
            es = empool.tile([c.ROWS, SD], F32, tag="es")
            nc.sync.dma_start(out=es[:], in_=rs_out[m][:, :])
            ea = empool.tile([c.ROWS, c.D], F32, tag="ea")
            nc.sync.dma_start(out=ea[:], in_=eao_d[m * c.ROWS:(m + 1) * c.ROWS, :])
            csc = empool.tile([c.ROWS, 1], F32, tag="csc")
            nc.sync.dma_start(out=csc[:], in_=cso_d[m * c.ROWS:(m + 1) * c.ROWS, :])

            cs08 = empool.tile([c.ROWS, 1], F32, tag="cs08")
            nc.vector.tensor_scalar_mul(cs08[:], csc[:], DECAY)
            csn = empool.tile([c.ROWS, 1], F32, tag="csn")
            nc.vector.scalar_tensor_tensor(
                out=csn[:], in0=es[:, c.D:SD], scalar=1.0 - DECAY, in1=cs08[:],
                op0=mybir.AluOpType.mult, op1=mybir.AluOpType.add,
            )
            ea08 = empool.tile([c.ROWS, c.D], F32, tag="ea08")
            nc.vector.tensor_scalar_mul(ea08[:], ea[:], DECAY)
            ean = empool.tile([c.ROWS, c.D], F32, tag="ean")
            nc.vector.scalar_tensor_tensor(
                out=ean[:], in0=es[:, :c.D], scalar=1.0 - DECAY, in1=ea08[:],
                op0=mybir.AluOpType.mult, op1=mybir.AluOpType.add,
            )
            smo = empool.tile([c.ROWS, 1], F32, tag="smo")
            nc.vector.scalar_tensor_tensor(
                out=smo[:], in0=csn[:], scalar=acol[:c.ROWS, :], in1=aepscol[:c.ROWS, :],
                op0=mybir.AluOpType.mult, op1=mybir.AluOpType.add,
            )
            sinv = empool.tile([c.ROWS, 1], F32, tag="sinv")
            nc.vector.reciprocal(sinv[:], smo[:])
            en = empool.tile([c.ROWS, c.D], F32, tag="en")
            nc.vector.tensor_scalar_mul(en[:], ean[:], sinv[:])

            nc.sync.dma_start(out=en_d[m * c.ROWS:(m + 1) * c.ROWS, :], in_=en[:])
            nc.sync.dma_start(out=csn_d[m * c.ROWS:(m + 1) * c.ROWS, :], in_=csn[:])
            nc.sync.dma_start(out=ean_d[m * c.ROWS:(m + 1) * c.ROWS, :], in_=ean[:])

    nc.compile()
    return nc


# ======================= host-side glue =======================

def owned_rows(cfg: Cfg, r: int) -> np.ndarray:
    """Global c indices owned by core r (concatenated per RS chunk)."""
    return np.concatenate([
        np.arange(m * cfg.CHUNK + r * cfg.ROWS, m * cfg.CHUNK + (r + 1) * cfg.ROWS)
        for m in range(cfg.NCHUNK)
    ])


def make_in_maps(cfg: Cfg, x, embed, cluster_size, embed_avg):
    """x: (n, d) full; embed: (C, D); cluster_size: (C,); embed_avg: (C, D)."""
    in_maps = []
    for r in range(cfg.CORES):
        rows = owned_rows(cfg, r)
        in_maps.append({
            "x_sh": np.ascontiguousarray(x[r * cfg.NTOK:(r + 1) * cfg.NTOK]),
            "embed": np.ascontiguousarray(embed),
            "cs_full": np.ascontiguousarray(cluster_size.reshape(cfg.C // P, P)),
            "cs_own": np.ascontiguousarray(cluster_size[rows, None]),
            "ea_own": np.ascontiguousarray(embed_avg[rows]),
        })
    return in_maps


def assemble(cfg: Cfg, results):
    """results: list per core of dict name->np.ndarray. Returns 6-tuple."""
    quantize = np.concatenate([results[r]["quantize_sh"] for r in range(cfg.CORES)], 0)
    embed_ind = np.concatenate([results[r]["embed_ind_sh"] for r in range(cfg.CORES)], 0)
    dist = np.concatenate([results[r]["dist_sh"] for r in range(cfg.CORES)], 0)
    embed_new = np.zeros((cfg.C, cfg.D), np.float32)
    cs_new = np.zeros((cfg.C,), np.float32)
    ea_new = np.zeros((cfg.C, cfg.D), np.float32)
    for r in range(cfg.CORES):
        rows = owned_rows(cfg, r)
        embed_new[rows] = results[r]["embed_new_sh"]
        cs_new[rows] = results[r]["cs_new_sh"][:, 0]
        ea_new[rows] = results[r]["ea_new_sh"]
    return (
        quantize[None], embed_ind[None], dist[None],
        embed_new[None], cs_new[None], ea_new[None],
    )


_CACHED = {}


def _get_nc(cfg: Cfg):
    key = (cfg.CORES, cfg.NTOK, cfg.C, cfg.D)
    if key not in _CACHED:
        _CACHED[key] = build_nc(cfg)
    return _CACHED[key]


def kernel(x, embed, cluster_size, embed_avg):
    """Full (unsharded) inputs with leading h=1 dim; returns the 6 outputs."""
    x = np.asarray(x)
    embed = np.asarray(embed)
    cluster_size = np.asarray(cluster_size)
    embed_avg = np.asarray(embed_avg)
    h, n, d = x.shape
    C = embed.shape[1]
    cfg = Cfg(cores=8, ntok=n // 8, C=C, D=d)
    nc = _get_nc(cfg)
    in_maps = make_in_maps(cfg, x[0], embed[0], cluster_size[0], embed_avg[0])
    res = bass_utils.run_bass_kernel_spmd(
        nc, in_maps, core_ids=list(range(cfg.CORES))
    )
    return assemble(cfg, res.results)


if __name__ == "__main__":
    cfg = Cfg()
    nc = build_nc(cfg)
    print("built ok:", len(nc.m.functions[0].allocations), "allocs")
